# revision 18
# baseline (speedup 1.0000x reference)
"""DeepIRT (DKVMN memory network) Trainium2 kernel.

Data-parallel over batch: 64 sequences / 8 cores = 8 per core.
The sequential memory update Mv <- Mv*(1 - w e^T) + w a^T is run exactly
with the DVE hardware scan primitive (tensor_tensor_scan):
  state[p] = A[p,t]*state[p] + B[p,t]   per partition p, along free dim t
with p = (m,d) state pairs: 25 tiles of [128 = 2m x 64d, t].
"""
import numpy as np
import ml_dtypes
import concourse.bass as bass
import concourse.bacc as bacc
import concourse.mybir as mybir
from concourse.tile import TileContext
from concourse.bass import IndirectOffsetOnAxis
from concourse.bass_utils import run_bass_kernel_spmd

F32 = mybir.dt.float32
BF16 = mybir.dt.bfloat16
I32 = mybir.dt.int32
AF = mybir.ActivationFunctionType
ALU = mybir.AluOpType

NUM_Q = 10000
B, L, M, D = 64, 2048, 50, 64
NCORES = 8
BLOC = B // NCORES            # 8 sequences per core
NTILE = (M + 1) // 2          # 25 scan tiles, each (2m x 64d)
TC = 512                      # time chunk
NCH = L // TC                 # 4 chunks
NTT = L // 128                # 16 token tiles per sequence

_BF = ml_dtypes.bfloat16


def _build_program():
    nc = bacc.Bacc("TRN2", target_bir_lowering=False)

    # ---- DRAM tensors ----
    qidx_d = nc.dram_tensor("qidx", [128, BLOC * NTT], I32, kind="ExternalInput")
    xidx_d = nc.dram_tensor("xidx", [128, BLOC * NTT], I32, kind="ExternalInput")
    kemb_d = nc.dram_tensor("k_emb", [NUM_Q, D], F32, kind="ExternalInput")
    vemb_d = nc.dram_tensor("v_emb", [2 * NUM_Q, D], F32, kind="ExternalInput")
    mkT_d = nc.dram_tensor("mkT", [D, M], BF16, kind="ExternalInput")
    we2_d = nc.dram_tensor("we2", [D + 1, D], BF16, kind="ExternalInput")
    wa2_d = nc.dram_tensor("wa2", [D + 1, D], BF16, kind="ExternalInput")
    wftop_d = nc.dram_tensor("wftop", [D + 1, D], BF16, kind="ExternalInput")
    wfbot_d = nc.dram_tensor("wfbot", [D, D], BF16, kind="ExternalInput")
    wab2_d = nc.dram_tensor("wab2", [D + 1, 1], BF16, kind="ExternalInput")
    wd2_d = nc.dram_tensor("wd2", [D + 1, 1], BF16, kind="ExternalInput")
    mvinit_d = nc.dram_tensor("mvinit", [128, NTILE], F32, kind="ExternalInput")
    selr_d = nc.dram_tensor("selr", [128, D], BF16, kind="ExternalInput")
    # per-sequence DRAM scratch for wT rows; the per-tile "wrep" broadcast is
    # a step-0-source DRAM->SBUF DMA on the HWDGE queue (row replicated to 64
    # partitions) instead of PE matmuls against a selection matrix.
    wtd = [nc.dram_tensor(f"wt_scratch_{b}", [M, L], BF16, kind="Internal")
           for b in range(BLOC)]
    ident_d = nc.dram_tensor("identity", [128, 128], F32, kind="ExternalInput")
    ones_d = nc.dram_tensor("ones_bf", [1, L], BF16, kind="ExternalInput")
    out_d = nc.dram_tensor("outp", [BLOC, L], F32, kind="ExternalOutput")

    with TileContext(nc) as tc:
        with tc.tile_pool(name="const", bufs=1) as cpool, \
             tc.tile_pool(name="seq", bufs=2) as spool, \
             tc.tile_pool(name="work", bufs=2) as wpool, \
             tc.tile_pool(name="psum", bufs=2, space="PSUM") as ppool, \
             tc.tile_pool(name="psumr", bufs=1, space="PSUM") as rpool:

            # ---- constants to SBUF ----
            qidx_sb = cpool.tile_from(qidx_d[:, :])
            xidx_sb = cpool.tile_from(xidx_d[:, :])
            mkT_sb = cpool.tile_from(mkT_d[:, :])
            we2_sb = cpool.tile_from(we2_d[:, :])
            wa2_sb = cpool.tile_from(wa2_d[:, :])
            wftop_sb = cpool.tile_from(wftop_d[:, :])
            wfbot_sb = cpool.tile_from(wfbot_d[:, :])
            wab2_sb = cpool.tile_from(wab2_d[:, :])
            wd2_sb = cpool.tile_from(wd2_d[:, :])
            mvinit_sb = cpool.tile_from(mvinit_d[:, :])
            selr_sb = cpool.tile_from(selr_d[:, :])
            ident = cpool.tile_from(ident_d[:, :])
            ones_sb = cpool.tile_from(ones_d[:, :])

            seqstate = {}

            def phase1(b):
                # gather + transpose
                kTx = spool.tile([D + 1, L], BF16, tag="kTx", name="kTx")
                vTx = spool.tile([D + 1, L], BF16, tag="vTx", name="vTx")
                nc.scalar.activation(kTx[D:D + 1, :], ones_sb[:, :], AF.Copy)
                nc.scalar.activation(vTx[D:D + 1, :], ones_sb[:, :], AF.Copy)
                kgall = wpool.tile([128, NTT * D], F32, tag="kgall", name="kgall")
                vgall = wpool.tile([128, NTT * D], F32, tag="vgall", name="vgall")
                for j in range(NTT):
                    c = b * NTT + j
                    nc.gpsimd.indirect_dma_start(
                        out=kgall[:, D * j:D * (j + 1)], out_offset=None,
                        in_=kemb_d[:, :],
                        in_offset=IndirectOffsetOnAxis(ap=qidx_sb[:, c:c + 1], axis=0))
                    nc.gpsimd.indirect_dma_start(
                        out=vgall[:, D * j:D * (j + 1)], out_offset=None,
                        in_=vemb_d[:, :],
                        in_offset=IndirectOffsetOnAxis(ap=xidx_sb[:, c:c + 1], axis=0))
                for j in range(NTT):
                    psK = ppool.tile([D, 128], F32, tag="big", name="psK")
                    nc.tensor.transpose(psK[:], kgall[:, D * j:D * (j + 1)], ident[:])
                    nc.scalar.activation(kTx[0:D, 128 * j:128 * (j + 1)], psK[:], AF.Copy)
                    psV = ppool.tile([D, 128], F32, tag="big", name="psV")
                    nc.tensor.transpose(psV[:], vgall[:, D * j:D * (j + 1)], ident[:])
                    nc.scalar.activation(vTx[0:D, 128 * j:128 * (j + 1)], psV[:], AF.Copy)

                # e2 / a2 (feature-major, stacked x2)
                e2 = spool.tile([128, L], BF16, tag="e2", name="e2")
                a2 = spool.tile([128, L], BF16, tag="a2", name="a2")
                for ch in range(NCH):
                    ts = slice(TC * ch, TC * (ch + 1))
                    psE = ppool.tile([D, TC], F32, tag="big", name="psE")
                    nc.tensor.matmul(psE[:], lhsT=we2_sb[:], rhs=vTx[:, ts],
                                     start=True, stop=True)
                    nc.scalar.activation(e2[0:D, ts], psE[:], AF.Sigmoid)
                    nc.scalar.activation(e2[D:128, ts], psE[:], AF.Sigmoid)
                    psA = ppool.tile([D, TC], F32, tag="big", name="psA")
                    nc.tensor.matmul(psA[:], lhsT=wa2_sb[:], rhs=vTx[:, ts],
                                     start=True, stop=True)
                    nc.scalar.activation(a2[0:D, ts], psA[:], AF.Tanh)
                    nc.scalar.activation(a2[D:128, ts], psA[:], AF.Tanh)

                # softmax -> w^T [50, L]
                wT = spool.tile([M, L], BF16, tag="wT", name="wT")
                for tt in range(NTT):
                    ts = slice(128 * tt, 128 * (tt + 1))
                    psL = ppool.tile([128, M], F32, tag="big", name="psL")
                    nc.tensor.matmul(psL[:], lhsT=kTx[0:D, ts], rhs=mkT_sb[:],
                                     start=True, stop=True)
                    expd = wpool.tile([128, M], F32, tag="expd", name="expd")
                    ssum = wpool.tile([128, 1], F32, tag="ssum", name="ssum")
                    # exp and its row-sum in one ACT op
                    nc.scalar.activation(expd[:], psL[:], AF.Exp,
                                         accum_out=ssum[:, 0:1])
                    rs = wpool.tile([128, 1], F32, tag="rs", name="rs")
                    nc.vector.reciprocal(rs[:], ssum[:])
                    wtok = wpool.tile([128, M], F32, tag="wtok", name="wtok")
                    nc.scalar.activation(wtok[:], expd[:], AF.Copy,
                                         scale=rs[:, 0:1])
                    psT = ppool.tile([M, 128], F32, tag="big", name="psT")
                    nc.tensor.transpose(psT[:], wtok[:], ident[:])
                    nc.scalar.activation(wT[:, ts], psT[:], AF.Copy)
                # stage wT in DRAM so phase2's per-tile broadcast can be an
                # indirect row-gather instead of PE matmuls + PSUM copies
                nc.sync.dma_start(wtd[b][:, :], wT[:, :])
                seqstate[b] = (kTx, vTx, e2, a2, wT)

            def phase23(b):
                kTx, vTx, e2, a2, wT = seqstate.pop(b)
                # ===== scan =====
                rTx = spool.tile([D + 1, L], BF16, tag="rTx", name="rTx")
                nc.scalar.activation(rTx[D:D + 1, :], ones_sb[:, :], AF.Copy)
                rp = [rpool.tile([D, TC], F32, tag=f"rp{c}", name=f"rp{c}")
                      for c in range(NCH)]
                def emit_read(prev_mvout, prev_wrep, i):
                    # z16 + read-reduce for tile i, deferred one iteration so
                    # ACT's At(i+1) latency hides behind DVE work.
                    z16 = wpool.tile([128, L], BF16, tag="z16", bufs=4,
                                     name="z16")
                    nc.vector.tensor_tensor(z16[:], prev_mvout[:, 0:L],
                                            prev_wrep[:], op=ALU.mult)
                    for ch in range(NCH):
                        ts = slice(TC * ch, TC * (ch + 1))
                        nc.tensor.matmul(rp[ch][:], lhsT=selr_sb[:],
                                         rhs=z16[:, ts],
                                         start=(i == 0), stop=(i == NTILE - 1))

                prev = None
                for i in range(NTILE):
                    mvout = spool.tile([128, L + 1], BF16, tag="mvout",
                                       bufs=3, name="mvout")
                    nc.scalar.activation(mvout[:, 0:1], mvinit_sb[:, i:i + 1], AF.Copy)
                    # broadcast w rows 2i,2i+1 across the tile's partitions via
                    # step-0-src DRAM->SBUF DMAs (HWDGE, off the gpsimd queue)
                    wrep16 = wpool.tile([128, L], BF16, tag="wrep16", bufs=4,
                                        name="wrep16")
                    nc.sync.dma_start(
                        out=wrep16[0:64, :],
                        in_=wtd[b][2 * i:2 * i + 1, :].broadcast_to((64, L)))
                    nc.sync.dma_start(
                        out=wrep16[64:128, :],
                        in_=wtd[b][2 * i + 1:2 * i + 2, :].broadcast_to((64, L)))
                    q1 = wpool.tile([128, L], BF16, tag="q1", bufs=4, name="q1")
                    nc.vector.tensor_tensor(q1[:], wrep16[:], e2[:, :],
                                            op=ALU.mult)
                    # At = 1 - q1 on ACT (DVE is the critical engine)
                    At = wpool.tile([128, L], BF16, tag="At", bufs=4, name="At")
                    nc.scalar.activation(At[:], q1[:], AF.Copy,
                                         bias=1.0, scale=-1.0)
                    # Bt on DVE, NOT gpsimd: gpsimd streaming locks the shared
                    # DVE<->GpSimd SBUF port and slows concurrent DVE ops 4x.
                    Bt = wpool.tile([128, L], BF16, tag="Bt", bufs=4, name="Bt")
                    nc.vector.tensor_tensor(Bt[:], wrep16[:], a2[:, :],
                                            op=ALU.mult)
                    if prev is not None:
                        emit_read(*prev)
                    nc.vector.tensor_tensor_scan(
                        out=mvout[:, 1:L + 1], data0=At[:], data1=Bt[:],
                        initial=mvinit_sb[:, i:i + 1],
                        op0=ALU.mult, op1=ALU.add)
                    prev = (mvout, wrep16, i)
                emit_read(*prev)
                for ch in range(NCH):
                    ts = slice(TC * ch, TC * (ch + 1))
                    nc.scalar.activation(rTx[0:D, ts], rp[ch][:], AF.Copy)

                # ===== output head =====
                outrow = spool.tile([1, L], F32, tag="outrow", name="outrow")
                for ch in range(NCH):
                    ts = slice(TC * ch, TC * (ch + 1))
                    psF = ppool.tile([D, TC], F32, tag="big", name="psF")
                    nc.tensor.matmul(psF[:], lhsT=wftop_sb[:], rhs=rTx[:, ts],
                                     start=True, stop=False)
                    nc.tensor.matmul(psF[:], lhsT=wfbot_sb[:], rhs=kTx[0:D, ts],
                                     start=False, stop=True)
                    fTx = wpool.tile([D + 1, TC], BF16, tag="fTx", name="fTx")
                    nc.scalar.activation(fTx[D:D + 1, :], ones_sb[:, 0:TC], AF.Copy)
                    nc.scalar.activation(fTx[0:D, :], psF[:], AF.Tanh)
                    psS = ppool.tile([1, TC], F32, tag="big", name="psS")
                    nc.tensor.matmul(psS[:], lhsT=wab2_sb[:], rhs=fTx[:],
                                     start=True, stop=True)
                    stu = wpool.tile([1, TC], F32, tag="stu", name="stu")
                    nc.scalar.activation(stu[:], psS[:], AF.Tanh)
                    psQ = ppool.tile([1, TC], F32, tag="big", name="psQ")
                    nc.tensor.matmul(psQ[:], lhsT=wd2_sb[:], rhs=kTx[:, ts],
                                     start=True, stop=True)
                    qdt = wpool.tile([1, TC], F32, tag="qdt", name="qdt")
                    nc.scalar.activation(qdt[:], psQ[:], AF.Tanh)
                    nc.vector.scalar_tensor_tensor(
                        out=outrow[0:1, ts], in0=stu[:], scalar=3.0,
                        in1=qdt[:], op0=ALU.mult, op1=ALU.subtract)
                nc.sync.dma_start(out_d[b:b + 1, :], outrow[:])

            # software-pipelined emission: phase1 runs one sequence ahead
            phase1(0)
            for b in range(BLOC):
                if b + 1 < BLOC:
                    phase1(b + 1)
                phase23(b)
    nc.compile()
    return nc


_prog_cache = {}


def _get_program():
    if "nc" not in _prog_cache:
        _prog_cache["nc"] = _build_program()
    return _prog_cache["nc"]


def _prep_host(question, response, k_emb, v_emb, Mk, Mv0, Wf, bf, We, be,
               Wa, ba, Wab, bab, Wd, bd):
    """Host-side input prep shared by all cores (weight layouts)."""
    bfc = lambda x: np.ascontiguousarray(np.asarray(x, np.float32)).astype(_BF)
    f32 = lambda x: np.ascontiguousarray(np.asarray(x, np.float32))

    consts = {}
    consts["k_emb"] = f32(k_emb)
    consts["v_emb"] = f32(v_emb)
    consts["mkT"] = bfc(np.asarray(Mk, np.float32).T)                  # [64, 50]
    consts["we2"] = bfc(np.concatenate([We, be[None, :]], 0))          # [65, 64]
    consts["wa2"] = bfc(np.concatenate([Wa, ba[None, :]], 0))
    consts["wftop"] = bfc(np.concatenate([Wf[:D], bf[None, :]], 0))    # [65, 64]
    consts["wfbot"] = bfc(Wf[D:])                                      # [64, 64]
    consts["wab2"] = bfc(np.concatenate([Wab, np.asarray(bab, np.float32).reshape(1, 1)], 0))
    consts["wd2"] = bfc(np.concatenate([Wd, np.asarray(bd, np.float32).reshape(1, 1)], 0))

    p = np.arange(128)
    mvinit = np.zeros((128, NTILE), np.float32)
    for i in range(NTILE):
        mvinit[:, i] = np.asarray(Mv0, np.float32)[
            np.minimum(2 * i + p // D, M - 1), p % D]
    consts["mvinit"] = mvinit



    selr = np.zeros((128, D), np.float32)
    selr[p, p % D] = 1.0
    consts["selr"] = selr.astype(_BF)
    consts["identity"] = np.eye(128, dtype=np.float32)
    consts["ones_bf"] = np.ones((1, L), np.float32).astype(_BF)
    return consts


def kernel(question, response, k_emb, v_emb, Mk, Mv0, Wf, bf, We, be,
           Wa, ba, Wab, bab, Wd, bd):
    question = np.asarray(question)
    response = np.asarray(response)
    consts = _prep_host(question, response, k_emb, v_emb, Mk, Mv0, Wf, bf,
                        We, be, Wa, ba, Wab, bab, Wd, bd)

    qi = question.astype(np.int64)
    xi = (qi + NUM_Q * np.asarray(response).astype(np.int64))
    in_maps = []
    for co in range(NCORES):
        sl = slice(BLOC * co, BLOC * (co + 1))
        qflat = qi[sl].reshape(-1).astype(np.int32)      # [16384]
        xflat = xi[sl].reshape(-1).astype(np.int32)
        m = dict(consts)
        m["qidx"] = np.ascontiguousarray(qflat.reshape(BLOC * NTT, 128).T)
        m["xidx"] = np.ascontiguousarray(xflat.reshape(BLOC * NTT, 128).T)
        in_maps.append(m)

    nc = _get_program()
    res = run_bass_kernel_spmd(nc, in_maps, list(range(NCORES)))
    out = np.concatenate([np.asarray(res.results[co]["outp"], np.float32)
                          for co in range(NCORES)], axis=0)
    return out


if __name__ == "__main__":
    d = dict(np.load("/tmp/inputs_full.npz"))
    out = kernel(**d)
    exp = np.load("/tmp/expected_np.npy")
    diff = np.abs(out - exp)
    print("max|diff|", diff.max(), "rel", diff.max() / np.abs(exp).max())

